# revision 10
# baseline (speedup 1.0000x reference)
"""Trainium2 Bass kernel for the GNN message-passing model.

Math (reference):
    base[b,s,t,j] = x[b,s,t,j]            (j<4)
    extra[b,s,t,c] = x[b,s,t,4+c]
    h_pre[b,c,s,h] = sum_{t,j} base[b,s,t,j]*mW1[5t+j,h]
                   + sum_t extra[b,s,t,c]*mW1[5t+4,h] + mb1[h]
    msg_sum[b,c,:] = sum_s relu(h_pre[b,c,s,:]) @ mW2 + N*mb2
    out = MLP(concat(msg_sum, x[:,:,-1,:4]))

Decomposition:
  * A[b,s,h] = base-part + mb1 (21 MFLOP of ~26 GFLOP) precomputed on host
    and folded into the device matmul as an extra contraction row against a
    ones-row in the rhs.
  * Per (b,s) pair the device does ONE bf16 matmul [K=11, M=128h, N=512c]
    producing h_pre for all columns.  Cost model: matmul time = out-free-size
    x 1 cycle/row for bf16, independent of K, so K=11 bf16 runs at ~213ns.
  * Pairs are processed in GROUPS of 2 sharing one 2-bank PSUM tile
    [128, 1024]; the relu/accumulate consumer handles both banks in one
    instruction, halving per-pair fixed costs.
  * sum_s relu(.) runs on THREE parallel lanes to spread the elementwise
    bottleneck (only ACT and DVE can read PSUM; Pool helps via ACT staging):
      lane A: ACT relu -> fp16 SBUF, PE PSUM-accumulates w2h.T @ r
      lane B: DVE fused hacc += max(psum, 0)      (fp16 wide accumulator)
      lane P: ACT relu -> fp16 SBUF, Pool tensor_tensor-adds into hacc3
    Tail per b: fold hacc/hacc3 through w2h on PE.
  * All four b-blocks accumulate into ONE PSUM bank (macc4 [128,512], b at
    partitions 32b..32b+32 via matmul tile_position), drained once at the
    end by a single ACT copy + DMA.
  * Sharding: data-parallel over the source axis s (512 -> 64 per core);
    each core produces partial msg[b,32,c] for all columns; the host sums
    the 8 partials and runs the tiny update MLP (0.15% of FLOPs) in numpy.
"""

import os
import numpy as np

import concourse.bass as bass
import concourse.mybir as mybir
from concourse.tile import TileContext
from concourse.bass_utils import run_bass_kernel_spmd

B, N, T, F = 4, 512, 10, 516
HID, MSG = 128, 32
NCORES = 8
SLOC = N // NCORES          # source rows per core
K1 = T + 1                  # 10 extra-feature rows + 1 ones-row (bias fold)
GP = 2                      # pairs per PSUM group (2 banks)
CH = 16                     # source rows per DMA chunk
F32 = mybir.dt.float32
BF16 = mybir.dt.bfloat16
FP16 = mybir.dt.float16

# lane pattern per 32 groups (one b-block): A = ACT relu + PE macc,
# B = DVE fused max+add, P = ACT relu + Pool add.  Engine balance target:
# A ~ 8, P ~ 9, B ~ 15 (PE/ACT/DVE/Pool all ~saturated).
def _make_pattern():
    pat = (["B", "A", "B", "P"] * 8)
    pat[24] = "P"            # 16B/8A/8P -> 15B/8A/9P
    return pat

PATTERN = _make_pattern()
DEFER = 2                   # groups of lookahead before A-lane maccs issue

_prog = None
last_results = None

# Tile emits semaphore waits for same-engine WAW/RAW deps (e.g. an ACT op
# waiting on the ACT sem for a pool buffer recycled from an older ACT write).
# Compute engines execute strictly in order, so these waits are redundant --
# and they overflow the 1-slot sync-wait budget of several ISA structs
# (ACTIVATE, TensorScalarPtr, MATMULT). Strip them post-scheduling.
_STRIP_TYPES = {
    "InstActivation", "InstTensorScalarPtr", "InstTensorTensor",
    "InstTensorCopy", "InstTensorReduce", "InstMatmult", "InstMemSet",
}
_ENG2SEM = None


def _strip_self_waits(nc):
    global _ENG2SEM
    if _ENG2SEM is None:
        _ENG2SEM = {
            mybir.EngineType.PE: "PE_",
            mybir.EngineType.Activation: "Activation_",
            mybir.EngineType.DVE: "DVE_",
            mybir.EngineType.Pool: "Pool_",
        }
    for fn in nc.m.functions:
        for blk in fn.blocks:
            for inst in blk.instructions:
                if type(inst).__name__ not in _STRIP_TYPES:
                    continue
                si = inst.sync_info
                if si is None or not si.on_wait:
                    continue
                pre = _ENG2SEM.get(inst.engine)
                if pre is None:
                    continue
                kept = [w for w in si.on_wait if not (w.ant_name or "").startswith(pre)]
                if len(kept) != len(si.on_wait):
                    si.on_wait = kept
    # Chunk-load DMAs: the WAR wait on the engine that read the recycled
    # buffer transitively dominates the WAW wait on the DMA that previously
    # filled it (that engine's reads each waited on that DMA themselves).
    eng_sems = ("PE_", "Activation_", "DVE_", "Pool_")
    for fn in nc.m.functions:
        for blk in fn.blocks:
            for inst in blk.instructions:
                if type(inst).__name__ != "InstDMACopy":
                    continue
                si = inst.sync_info
                if si is None or not si.on_wait:
                    continue
                has_eng = any((w.ant_name or "").startswith(eng_sems) for w in si.on_wait)
                if not has_eng:
                    continue
                kept = [
                    w for w in si.on_wait
                    if not (w.ant_name or "").startswith(("DMAHW", "DMASW"))
                ]
                if len(kept) != len(si.on_wait):
                    si.on_wait = kept
    # Kernel-tail Drain: waits on every DMA queue overflow the CTRL struct's
    # wait budget. Input-DMA waits are dominated by the engine waits (each
    # load was read by a compute engine before the drain); only the queues
    # carrying the output DMAs must be waited on directly.
    out_sems = set()
    for fn in nc.m.functions:
        for blk in fn.blocks:
            for inst in blk.instructions:
                if type(inst).__name__ != "InstDMACopy":
                    continue
                outs = getattr(inst, "outs", None) or []
                to_dram = any("msg_out" in (getattr(o, "memref", "") or "")
                              for o in outs)
                si = inst.sync_info
                if to_dram and si and si.on_update:
                    for u in si.on_update:
                        out_sems.add(u.ant_name)
    drain_split = 0
    for fn in nc.m.functions:
        for blk in fn.blocks:
            for ii in range(len(blk.instructions)):
                inst = blk.instructions[ii]
                if type(inst).__name__ != "InstDrain":
                    continue
                si = inst.sync_info
                if si is None or not si.on_wait or len(si.on_wait) <= 1:
                    continue
                waits = [
                    w for w in si.on_wait
                    if not (w.ant_name or "").startswith(("DMAHW", "DMASW"))
                    or w.ant_name in out_sems
                ]
                # split into a chain of drains with one wait each (the SP
                # CTRL struct has a single sync-wait slot)
                pre = []
                while len(waits) > 1:
                    chunk, waits = waits[:1], waits[1:]
                    d = mybir.InstDrain(
                        name=f"{inst.name}_split{drain_split}", ins=[], outs=[],
                        sync_info=mybir.SyncInfo(on_wait=chunk, on_update=[]),
                    )
                    d.engine = inst.engine
                    drain_split += 1
                    pre.append(d)
                si.on_wait = waits
                for d in reversed(pre):
                    blk.instructions.insert(ii, d)
                break


def _build_program():
    nc = bass.Bass(trn_type="TRN2")
    # packed input: per (b, s) an [K1, N+HID] block -- first N cols are the
    # matmul rhs (extra features + ones row), last HID cols the per-pair lhsT
    # (W1x rows + folded bias/A row).  One tensor -> one DMA sem per chunk.
    ext = nc.dram_tensor("ext", [B, SLOC, K1, N + HID], BF16, kind="ExternalInput")
    w2h_d = nc.dram_tensor("w2h", [HID, MSG], FP16, kind="ExternalInput")
    msg_out = nc.dram_tensor("msg_out", [4 * MSG, N], F32, kind="ExternalOutput")

    GPB = SLOC // GP            # groups per b-block (32)
    GPC = CH // GP              # groups per chunk (8)

    with TileContext(nc) as tc:
        with (
            tc.tile_pool(name="const", bufs=1) as constp,
            tc.tile_pool(name="big", bufs=2) as bigp,
            tc.tile_pool(name="relu", bufs=4) as rp,     # ACT-relu'd fp16 tiles
            tc.tile_pool(name="haccs", bufs=1) as hp,    # wide accumulators
            tc.tile_pool(name="out", bufs=1) as outp,
            tc.tile_pool(name="ps", bufs=3, space="PSUM") as pp,
            tc.tile_pool(name="pwarm", bufs=1, space="PSUM") as pwp,
            tc.tile_pool(name="pacc", bufs=1, space="PSUM") as pa,
        ):
            w2h = constp.tile([HID, MSG], FP16)
            nc.sync.dma_start(w2h[:], w2h_d[:])
            # warmup touch of w2h on PE so later macc matmuls don't need a
            # DMA wait on top of their relu-tile wait
            warm = pwp.tile([MSG, 1], F32, tag="warm")
            nc.tensor.matmul(warm[:], w2h[:], w2h[:, :1], start=True, stop=True)

            macc4 = pa.tile([4 * MSG, N], F32, tag="macc4")

            for b in range(B):
                pos = (0, 32 * b) if b >= 3 else None
                blk = macc4[b * MSG:(b + 1) * MSG, :]
                hacc = None          # DVE wide accumulator (fp16)
                hacc3 = None         # Pool wide accumulator (fp16)
                nmm = 0              # maccs issued into blk so far
                deferq = []          # deferred PE macc thunks

                def macc(rhs_ap, stop=False, _pos=pos, _blk=blk):
                    nonlocal nmm
                    kw = {"tile_position": _pos} if _pos else {}
                    nc.tensor.matmul(
                        _blk, w2h[:], rhs_ap,
                        start=(nmm == 0), stop=stop,
                        skip_group_check=True, **kw,
                    )
                    nmm += 1

                for g in range(SLOC // CH):
                    big_t = bigp.tile([K1, CH, N + HID], BF16, tag="big",
                                      name=f"big{b}_{g}")
                    nc.sync.dma_start(
                        big_t[:],
                        ext[b, g * CH:(g + 1) * CH].rearrange("s k c -> k s c"),
                    )
                    # tiny PE touch of the fresh chunk: absorbs the DMA wait
                    # so real matmuls carry only the PSUM-recycle wait
                    wt = pwp.tile([MSG, 1], F32, tag="warm", name=f"wt{b}_{g}")
                    nc.tensor.matmul(
                        wt[:, :1], big_t[:, 0, :MSG], big_t[:, 0, :1],
                        start=True, stop=True,
                    )
                    for gi in range(GPC):
                        grp = g * GPC + gi
                        ty = PATTERN[grp % len(PATTERN)]
                        ps = pp.tile([HID, GP * N], F32, tag="ps",
                                     name=f"ps{b}_{grp}")
                        for j in range(GP):
                            si = (grp * GP + j) % CH
                            nc.tensor.matmul(
                                ps[:, j * N:(j + 1) * N],
                                big_t[:, si, N:N + HID], big_t[:, si, :N],
                                start=True, stop=True,
                            )
                        if ty == "A":
                            r = rp.tile([HID, GP * N], FP16, tag="rA",
                                        name=f"rA{b}_{grp}")
                            nc.scalar.activation(
                                r[:], ps[:], mybir.ActivationFunctionType.Relu
                            )
                            deferq.append(
                                (grp, lambda r=r: (macc(r[:, :N]),
                                                   macc(r[:, N:]))))
                        elif ty == "P":
                            # unique tile per P-group: a recycled slot would
                            # put a Pool-sem WAR wait on this ACT relu, which
                            # overflows the 1-slot ACTIVATE sync budget
                            r = rp.tile([HID, GP * N], FP16,
                                        tag=f"rP{b}_{grp}",
                                        name=f"rP{b}_{grp}", bufs=1)
                            nc.scalar.activation(
                                r[:], ps[:], mybir.ActivationFunctionType.Relu
                            )
                            if hacc3 is None:
                                hacc3 = hp.tile([HID, GP * N], FP16,
                                                tag=f"h3_{b}",
                                                name=f"h3_{b}")
                                nc.gpsimd.tensor_copy(hacc3[:], r[:])
                            else:
                                nc.gpsimd.tensor_tensor(
                                    hacc3[:], r[:], hacc3[:],
                                    op=mybir.AluOpType.add,
                                )
                        else:   # "B": DVE fused relu+accumulate from PSUM
                            if hacc is None:
                                hacc = hp.tile([HID, GP * N], FP16,
                                               tag=f"hb_{b}",
                                               name=f"hb_{b}")
                                nc.vector.tensor_scalar(
                                    hacc[:], ps[:], 0.0, None,
                                    op0=mybir.AluOpType.max,
                                )
                            else:
                                nc.vector.scalar_tensor_tensor(
                                    hacc[:], ps[:], 0.0, hacc[:],
                                    op0=mybir.AluOpType.max,
                                    op1=mybir.AluOpType.add,
                                )
                        # release deferred maccs DEFER groups later
                        while deferq and deferq[0][0] <= grp - DEFER:
                            deferq.pop(0)[1]()
                for _, fn_ in deferq:
                    fn_()
                # fold the DVE/Pool accumulators through w2h as well; the
                # final fold closes this b-block's accumulation group
                tails = []
                if hacc is not None:
                    tails += [hacc[:, :N], hacc[:, N:]]
                if hacc3 is not None:
                    tails += [hacc3[:, :N], hacc3[:, N:]]
                for i, ap in enumerate(tails):
                    macc(ap, stop=(i == len(tails) - 1))

            # drain the stacked macc bank once: ACT copy + single DMA
            ot = outp.tile([4 * MSG, N], F32)
            nc.scalar.copy(ot[:], macc4[:])
            nc.sync.dma_start(msg_out[:], ot[:])
    _strip_self_waits(nc)
    return nc


def _get_prog():
    global _prog
    if _prog is None:
        _prog = _build_program()
    return _prog


def kernel(x, mW1, mb1, mW2, mb2, iW1, ib1, iW2, ib2):
    global last_results
    import ml_dtypes
    x = np.ascontiguousarray(np.asarray(x, dtype=np.float32))
    mW1 = np.asarray(mW1, dtype=np.float32)
    mb1 = np.asarray(mb1, dtype=np.float32)
    mW2 = np.ascontiguousarray(np.asarray(mW2, dtype=np.float32))
    mb2 = np.asarray(mb2, dtype=np.float32)

    # host prep: A[b,s,h] = base_flat @ W1b + mb1 (tiny), weight slices
    base = x[:, :, :, :4]                                  # [B,N,T,4]
    base_flat = base.reshape(B, N, T * 4)
    W1b = mW1.reshape(T, 5, HID)[:, :4, :].reshape(T * 4, HID)
    W1x = np.ascontiguousarray(mW1.reshape(T, 5, HID)[:, 4, :])   # [T,HID]
    A = base_flat @ W1b + mb1                              # [B,N,HID]

    bf16 = ml_dtypes.bfloat16
    w2h = mW2.astype(np.float16)

    in_maps = []
    for k in range(NCORES):
        sl = slice(k * SLOC, (k + 1) * SLOC)
        ext_k = np.empty((B, SLOC, K1, N + HID), dtype=np.float32)
        # rhs: rows 0..T-1 = extra features (c along cols), row T = ones
        ext_k[:, :, :T, :N] = np.transpose(x[:, sl, :, 4:4 + N], (0, 1, 2, 3))
        ext_k[:, :, T, :N] = 1.0
        # lhsT: rows 0..T-1 = W1x, row T = A
        ext_k[:, :, :T, N:] = W1x[None, None, :, :]
        ext_k[:, :, T, N:] = A[:, sl, :]
        in_maps.append({
            "ext": np.ascontiguousarray(ext_k.astype(bf16)),
            "w2h": w2h,
        })

    nc = _get_prog()
    trace = bool(int(os.environ.get("KERNEL_TRACE", "0")))
    try:
        res = run_bass_kernel_spmd(
            nc, in_maps, core_ids=list(range(NCORES)), trace=trace,
        )
    except ModuleNotFoundError:
        # axon NTFF profiling hook unavailable -> rerun without trace
        res = run_bass_kernel_spmd(
            nc, in_maps, core_ids=list(range(NCORES)), trace=False,
        )
    last_results = res

    msg_part = np.zeros((4 * MSG, N), dtype=np.float32)
    for r in res.results:
        msg_part += r["msg_out"]
    msg_part = msg_part.reshape(B, MSG, N)

    msg_sum = np.transpose(msg_part, (0, 2, 1)) + N * mb2  # [B,N,MSG]
    node_feat = x[:, :, -1, :4]
    mi = np.concatenate([msg_sum, node_feat], axis=-1)     # [B,N,MSG+4]
    h2 = np.maximum(mi @ np.asarray(iW1, dtype=np.float32)
                    + np.asarray(ib1, dtype=np.float32), 0.0)
    out = h2 @ np.asarray(iW2, dtype=np.float32) + np.asarray(ib2, dtype=np.float32)
    return out.astype(np.float32)


# revision 29
# speedup vs baseline: 1.0942x; 1.0942x over previous
"""Trainium2 Bass kernel for the GNN message-passing model.

Math (reference):
    base[b,s,t,j] = x[b,s,t,j]            (j<4)
    extra[b,s,t,c] = x[b,s,t,4+c]
    h_pre[b,c,s,h] = sum_{t,j} base[b,s,t,j]*mW1[5t+j,h]
                   + sum_t extra[b,s,t,c]*mW1[5t+4,h] + mb1[h]
    msg_sum[b,c,:] = sum_s relu(h_pre[b,c,s,:]) @ mW2 + N*mb2
    out = MLP(concat(msg_sum, x[:,:,-1,:4]))

Decomposition:
  * A[b,s,h] = base-part + mb1 (21 MFLOP of ~26 GFLOP) precomputed on host
    and folded into the device matmul as an extra contraction row against a
    ones-row in the rhs.
  * Per (b,s) pair the device does ONE bf16 matmul [K=11, M=128h, N=512c]
    producing h_pre for all columns.  Cost model: matmul time = out-free-size
    x 1 cycle/row for bf16, independent of K, so K=11 bf16 runs at ~213ns.
  * Pairs are processed in GROUPS of 2 sharing one 2-bank PSUM tile
    [128, 1024]; the relu/accumulate consumer handles both banks in one
    instruction, halving per-pair fixed costs.
  * sum_s relu(.) runs on THREE parallel lanes to spread the elementwise
    bottleneck (only ACT and DVE can read PSUM; Pool helps via ACT staging):
      lane A: ACT relu -> fp16 SBUF, PE PSUM-accumulates w2h.T @ r
      lane B: DVE fused hacc += max(psum, 0)      (fp16 wide accumulator)
      lane P: ACT relu -> fp16 SBUF, Pool tensor_tensor-adds into hacc3
    Tail per b: fold hacc/hacc3 through w2h on PE.
  * All four b-blocks accumulate into ONE PSUM bank (macc4 [128,512], b at
    partitions 32b..32b+32 via matmul tile_position), drained once at the
    end by a single ACT copy + DMA.
  * Sharding: data-parallel over the source axis s (512 -> 64 per core);
    each core produces partial msg[b,32,c] for all columns; the host sums
    the 8 partials and runs the tiny update MLP (0.15% of FLOPs) in numpy.
"""

import os
import numpy as np

import concourse.bass as bass
import concourse.mybir as mybir
from concourse.tile import TileContext
from concourse.bass_utils import run_bass_kernel_spmd

B, N, T, F = 4, 512, 10, 516
HID, MSG = 128, 32
NCORES = 8
SLOC = N // NCORES          # source rows per core
K1 = T + 1                  # 10 extra-feature rows + 1 ones-row (bias fold)
GP = 2                      # pairs per PSUM group (2 banks)
CH = 16                     # source rows per DMA chunk
F32 = mybir.dt.float32
BF16 = mybir.dt.bfloat16
FP16 = mybir.dt.float16

# lane pattern per 32 groups (one b-block): A = ACT relu + PE macc,
# B = DVE fused max+add, P = ACT relu + Pool add.  Engine balance target:
# A ~ 8, P ~ 9, B ~ 15 (PE/ACT/DVE/Pool all ~saturated).  P groups are
# front-loaded so the slow Pool lane gets a long runway and the b-tail only
# drains the short DVE/ACT chains.
def _make_pattern():
    # front phase: consumer-paced P/B alternation (PE leads, absorbing the
    # PSUM-recycle latency); back phase: PE-paced A-groups.
    return (["P", "B"] * 9) + (["A", "B"] * 6) + ["A", "A"]

PATTERN = _make_pattern()
DEFER = 2                   # groups of lookahead before A-lane maccs issue
TAILDEF = 6                 # groups into the next b before tail maccs issue

_prog = None
last_results = None

# Tile emits semaphore waits for same-engine WAW/RAW deps (e.g. an ACT op
# waiting on the ACT sem for a pool buffer recycled from an older ACT write).
# Compute engines execute strictly in order, so these waits are redundant --
# and they overflow the 1-slot sync-wait budget of several ISA structs
# (ACTIVATE, TensorScalarPtr, MATMULT). Strip them post-scheduling.
_STRIP_TYPES = {
    "InstActivation", "InstTensorScalarPtr", "InstTensorTensor",
    "InstTensorCopy", "InstTensorReduce", "InstMatmult", "InstMemSet",
}
_ENG2SEM = None


def _strip_self_waits(nc):
    global _ENG2SEM
    if _ENG2SEM is None:
        _ENG2SEM = {
            mybir.EngineType.PE: "PE_",
            mybir.EngineType.Activation: "Activation_",
            mybir.EngineType.DVE: "DVE_",
            mybir.EngineType.Pool: "Pool_",
        }
    for fn in nc.m.functions:
        for blk in fn.blocks:
            for inst in blk.instructions:
                if type(inst).__name__ not in _STRIP_TYPES:
                    continue
                si = inst.sync_info
                if si is None or not si.on_wait:
                    continue
                pre = _ENG2SEM.get(inst.engine)
                if pre is None:
                    continue
                kept = [w for w in si.on_wait if not (w.ant_name or "").startswith(pre)]
                if len(kept) != len(si.on_wait):
                    si.on_wait = kept
    # Chunk-load DMAs: the WAR wait on the engine that read the recycled
    # buffer transitively dominates the WAW wait on the DMA that previously
    # filled it (that engine's reads each waited on that DMA themselves).
    eng_sems = ("PE_", "Activation_", "DVE_", "Pool_")
    for fn in nc.m.functions:
        for blk in fn.blocks:
            for inst in blk.instructions:
                if type(inst).__name__ != "InstDMACopy":
                    continue
                si = inst.sync_info
                if si is None or not si.on_wait:
                    continue
                has_eng = any((w.ant_name or "").startswith(eng_sems) for w in si.on_wait)
                if not has_eng:
                    continue
                kept = [
                    w for w in si.on_wait
                    if not (w.ant_name or "").startswith(("DMAHW", "DMASW"))
                ]
                if len(kept) != len(si.on_wait):
                    si.on_wait = kept
    # Kernel-tail Drain: waits on every DMA queue overflow the CTRL struct's
    # wait budget. Input-DMA waits are dominated by the engine waits (each
    # load was read by a compute engine before the drain); only the queues
    # carrying the output DMAs must be waited on directly.
    out_sems = set()
    for fn in nc.m.functions:
        for blk in fn.blocks:
            for inst in blk.instructions:
                if type(inst).__name__ != "InstDMACopy":
                    continue
                outs = getattr(inst, "outs", None) or []
                to_dram = any("msg_out" in (getattr(o, "memref", "") or "")
                              for o in outs)
                si = inst.sync_info
                if to_dram and si and si.on_update:
                    for u in si.on_update:
                        out_sems.add(u.ant_name)
    drain_split = 0
    for fn in nc.m.functions:
        for blk in fn.blocks:
            for ii in range(len(blk.instructions)):
                inst = blk.instructions[ii]
                if type(inst).__name__ != "InstDrain":
                    continue
                si = inst.sync_info
                if si is None or not si.on_wait or len(si.on_wait) <= 1:
                    continue
                waits = [
                    w for w in si.on_wait
                    if not (w.ant_name or "").startswith(("DMAHW", "DMASW"))
                    or w.ant_name in out_sems
                ]
                # split into a chain of drains with one wait each (the SP
                # CTRL struct has a single sync-wait slot)
                pre = []
                while len(waits) > 1:
                    chunk, waits = waits[:1], waits[1:]
                    d = mybir.InstDrain(
                        name=f"{inst.name}_split{drain_split}", ins=[], outs=[],
                        sync_info=mybir.SyncInfo(on_wait=chunk, on_update=[]),
                    )
                    d.engine = inst.engine
                    drain_split += 1
                    pre.append(d)
                si.on_wait = waits
                for d in reversed(pre):
                    blk.instructions.insert(ii, d)
                break


def _build_program():
    nc = bass.Bass(trn_type="TRN2")
    # packed input: per (b, s) an [K1, N+HID] block -- first N cols are the
    # matmul rhs (extra features + ones row), last HID cols the per-pair lhsT
    # (W1x rows + folded bias/A row).  One tensor -> one DMA sem per chunk.
    ext = nc.dram_tensor("ext", [B, SLOC, K1, N + HID], BF16, kind="ExternalInput")
    w2h_d = nc.dram_tensor("w2h", [HID, MSG], FP16, kind="ExternalInput")
    msg_out = nc.dram_tensor("msg_out", [4 * MSG, N], F32, kind="ExternalOutput")

    GPB = SLOC // GP            # groups per b-block (32)
    GPC = CH // GP              # groups per chunk (8)

    with TileContext(nc) as tc:
        with (
            tc.tile_pool(name="const", bufs=1) as constp,
            tc.tile_pool(name="big", bufs=2) as bigp,
            tc.tile_pool(name="relu", bufs=4) as rp,     # ACT-relu'd fp16 tiles
            tc.tile_pool(name="haccs", bufs=1) as hp,    # wide accumulators
            tc.tile_pool(name="out", bufs=1) as outp,
            tc.tile_pool(name="ps", bufs=3, space="PSUM") as pp,
            tc.tile_pool(name="pwarm", bufs=1, space="PSUM") as pwp,
            tc.tile_pool(name="pacc", bufs=1, space="PSUM") as pa,
        ):
            w2h = constp.tile([HID, MSG], FP16)
            nc.sync.dma_start(w2h[:], w2h_d[:])
            # warmup touch of w2h on PE so later macc matmuls don't need a
            # DMA wait on top of their relu-tile wait
            warm = pwp.tile([MSG, 1], F32, tag="warm")
            nc.tensor.matmul(warm[:], w2h[:], w2h[:, :1], start=True, stop=True)

            macc4 = pa.tile([4 * MSG, N], F32, tag="macc4")
            ot = outp.tile([4 * MSG, N], F32)

            nmm = [0] * B            # maccs issued per b-block
            deferq = []              # (due_global_group, thunk) min-heap-ish

            def macc(bb, rhs_ap, stop=False):
                pos = (0, 32 * bb) if bb >= 3 else None
                kw = {"tile_position": pos} if pos else {}
                nc.tensor.matmul(
                    macc4[bb * MSG:(bb + 1) * MSG, :], w2h[:], rhs_ap,
                    start=(nmm[bb] == 0), stop=stop,
                    skip_group_check=True, **kw,
                )
                nmm[bb] += 1

            def flush_defer(now):
                while deferq and deferq[0][0] <= now:
                    deferq.pop(0)[1]()

            for b in range(B):
                hacc = None          # DVE wide accumulator (fp16)
                haccD = None         # DMA-accumulated wide accumulator (fp16)

                chunks = [(o, CH) for o in range(0, SLOC, CH)]
                for g, (s0, clen) in enumerate(chunks):
                    big_t = bigp.tile([K1, clen, N + HID], BF16,
                                      tag=f"big{clen}",
                                      name=f"big{b}_{g}")
                    nc.sync.dma_start(
                        big_t[:],
                        ext[b, s0:s0 + clen].rearrange("s k c -> k s c"),
                    )
                    # tiny PE touch of the fresh chunk: absorbs the DMA wait
                    # so real matmuls carry only the PSUM-recycle wait
                    wt = pwp.tile([MSG, 1], F32, tag="warm", name=f"wt{b}_{g}")
                    nc.tensor.matmul(
                        wt[:, :1], big_t[:, 0, :MSG], big_t[:, 0, :1],
                        start=True, stop=True,
                    )
                    for gi in range(clen // GP):
                        grp = s0 // GP + gi
                        gglob = b * GPB + grp
                        ty = PATTERN[grp % len(PATTERN)]
                        ps = pp.tile([HID, GP * N], F32, tag="ps",
                                     name=f"ps{b}_{grp}")
                        for j in range(GP):
                            si = gi * GP + j
                            nc.tensor.matmul(
                                ps[:, j * N:(j + 1) * N],
                                big_t[:, si, N:N + HID], big_t[:, si, :N],
                                start=True, stop=True,
                            )
                        if ty == "A":
                            r = rp.tile([HID, GP * N], FP16, tag="rA",
                                        name=f"rA{b}_{grp}")
                            nc.scalar.activation(
                                r[:], ps[:], mybir.ActivationFunctionType.Relu
                            )
                            deferq.append(
                                (gglob + DEFER,
                                 lambda r=r, bb=b: (macc(bb, r[:, :N]),
                                                    macc(bb, r[:, N:]))))
                        elif ty == "P":
                            # unique tile per P-group: a recycled slot would
                            # put a Pool-sem WAR wait on this ACT relu, which
                            # overflows the 1-slot ACTIVATE sync budget
                            r = rp.tile([HID, GP * N], FP16,
                                        tag=f"rP{b}_{grp}",
                                        name=f"rP{b}_{grp}", bufs=1)
                            nc.scalar.activation(
                                r[:], ps[:], mybir.ActivationFunctionType.Relu
                            )
                            if haccD is None:
                                haccD = hp.tile([HID, GP * N], FP16,
                                                tag=f"h3_{b}",
                                                name=f"h3_{b}")
                                nc.gpsimd.tensor_copy(haccD[:], r[:])
                            else:
                                nc.gpsimd.tensor_tensor(
                                    haccD[:], r[:], haccD[:],
                                    op=mybir.AluOpType.add,
                                )
                        else:   # "B": DVE fused relu+accumulate from PSUM
                            if hacc is None:
                                hacc = hp.tile([HID, GP * N], FP16,
                                               tag=f"hb_{b}",
                                               name=f"hb_{b}")
                                nc.vector.tensor_scalar(
                                    hacc[:], ps[:], 0.0, None,
                                    op0=mybir.AluOpType.max,
                                )
                            else:
                                nc.vector.scalar_tensor_tensor(
                                    hacc[:], ps[:], 0.0, hacc[:],
                                    op0=mybir.AluOpType.max,
                                    op1=mybir.AluOpType.add,
                                )
                        # release deferred work whose due group has arrived
                        flush_defer(gglob)

                # tail: fold the DVE/Pool accumulators through w2h, close the
                # b-block's accumulation group, then drain it to SBUF.
                # Deferred into the next b's stream so PE keeps streaming
                # while DVE/Pool finish this b.
                def tail(bb=b, _hacc=hacc, _hacc3=haccD):
                    aps = [_hacc[:, :N], _hacc[:, N:],
                           _hacc3[:, :N], _hacc3[:, N:]]
                    for i, ap in enumerate(aps):
                        macc(bb, ap, stop=(i == len(aps) - 1))
                    # drain finished halves early (partition offsets must stay
                    # in {0, 32, 64} for AP slicing)
                    if bb == 1:
                        nc.scalar.copy(ot[0:64, :], macc4[0:64, :])
                    elif bb == 3:
                        nc.scalar.copy(ot[64:128, :], macc4[64:128, :])
                deferq.append(((b + 1) * GPB + TAILDEF, tail))

            flush_defer(B * GPB + GPB)
            nc.sync.dma_start(msg_out[:], ot[:])
    _strip_self_waits(nc)
    return nc


def _get_prog():
    global _prog
    if _prog is None:
        _prog = _build_program()
    return _prog


def kernel(x, mW1, mb1, mW2, mb2, iW1, ib1, iW2, ib2):
    global last_results
    import ml_dtypes
    x = np.ascontiguousarray(np.asarray(x, dtype=np.float32))
    mW1 = np.asarray(mW1, dtype=np.float32)
    mb1 = np.asarray(mb1, dtype=np.float32)
    mW2 = np.ascontiguousarray(np.asarray(mW2, dtype=np.float32))
    mb2 = np.asarray(mb2, dtype=np.float32)

    # host prep: A[b,s,h] = base_flat @ W1b + mb1 (tiny), weight slices
    base = x[:, :, :, :4]                                  # [B,N,T,4]
    base_flat = base.reshape(B, N, T * 4)
    W1b = mW1.reshape(T, 5, HID)[:, :4, :].reshape(T * 4, HID)
    W1x = np.ascontiguousarray(mW1.reshape(T, 5, HID)[:, 4, :])   # [T,HID]
    A = base_flat @ W1b + mb1                              # [B,N,HID]

    bf16 = ml_dtypes.bfloat16
    w2h = mW2.astype(np.float16)

    in_maps = []
    for k in range(NCORES):
        sl = slice(k * SLOC, (k + 1) * SLOC)
        ext_k = np.empty((B, SLOC, K1, N + HID), dtype=np.float32)
        # rhs: rows 0..T-1 = extra features (c along cols), row T = ones
        ext_k[:, :, :T, :N] = np.transpose(x[:, sl, :, 4:4 + N], (0, 1, 2, 3))
        ext_k[:, :, T, :N] = 1.0
        # lhsT: rows 0..T-1 = W1x, row T = A
        ext_k[:, :, :T, N:] = W1x[None, None, :, :]
        ext_k[:, :, T, N:] = A[:, sl, :]
        in_maps.append({
            "ext": np.ascontiguousarray(ext_k.astype(bf16)),
            "w2h": w2h,
        })

    nc = _get_prog()
    trace = bool(int(os.environ.get("KERNEL_TRACE", "0")))
    try:
        res = run_bass_kernel_spmd(
            nc, in_maps, core_ids=list(range(NCORES)), trace=trace,
        )
    except ModuleNotFoundError:
        # axon NTFF profiling hook unavailable -> rerun without trace
        res = run_bass_kernel_spmd(
            nc, in_maps, core_ids=list(range(NCORES)), trace=False,
        )
    last_results = res

    msg_part = np.zeros((4 * MSG, N), dtype=np.float32)
    for r in res.results:
        msg_part += r["msg_out"]
    msg_part = msg_part.reshape(B, MSG, N)

    msg_sum = np.transpose(msg_part, (0, 2, 1)) + N * mb2  # [B,N,MSG]
    node_feat = x[:, :, -1, :4]
    mi = np.concatenate([msg_sum, node_feat], axis=-1)     # [B,N,MSG+4]
    h2 = np.maximum(mi @ np.asarray(iW1, dtype=np.float32)
                    + np.asarray(ib1, dtype=np.float32), 0.0)
    out = h2 @ np.asarray(iW2, dtype=np.float32) + np.asarray(ib2, dtype=np.float32)
    return out.astype(np.float32)
